# revision 2
# baseline (speedup 1.0000x reference)
"""AdjustHueSaturation Trainium2 kernel.

Full inputs: imgs (64,3,512,512) f32 in [0,1], xform_params (64,2) f32
(hue delta in [-0.5,0.5], sat scale in [0.2,2]).
Output: (64,3,512,512) f32.

Strategy: pure batch data-parallel across 8 NeuronCores (8 images/core).
Per-pixel math (no divisions, no 6-way select):
    maxc, minc, cr = max, min, chroma
    icr  = exp(-ln(cr + 1e-30))             # 1/cr via ACT LUTs (safe at cr=0)
    c    = min(cr*ds, maxc)                 # = v * clip(s*ds,0,1)
    num  = cr*(6dh+6)  + (g-b)  overwritten by
           cr*(6dh+8)  + (b-r)  where g==maxc, then by
           cr*(6dh+10) + (r-g)  where b==maxc     # priority select r>g>b
    u    = num*icr          # = hue*6 + 6dh + 6 in [2,14]
    m    = mod(u, 6)        # in [0,6)
    out_ch = v - clamp(c*(|m-k|-1), 0, c)  (sign-flipped for red):
       r: (v-c) + clamp(c*(|m-3|-1), 0, c)
       g:  v    - clamp(c*(|m-2|-1), 0, c)
       b:  v    - clamp(c*(|m-4|-1), 0, c)
Matches the reference exactly up to fp rounding (continuous at all branch
boundaries, including exact channel ties; hue garbage at cr~0 is masked
by c~0).
"""

import numpy as np

B, C, H, W = 64, 3, 512, 512
N_CORES = 8
IPC = B // N_CORES          # images per core
P = 128                     # SBUF partitions
FDTOT = (H * W) // P        # 2048 f32 per partition per plane
FD = 512                    # free-dim chunk per tile
NCH = FDTOT // FD

POOL_MOD = False             # compute mod on GpSimd (else emulate on DVE)

_nc_cache = {}


def _build_nc():
    from concourse import bass, bacc, mybir
    from concourse.tile import TileContext

    f32 = mybir.dt.float32
    Alu = mybir.AluOpType
    Act = mybir.ActivationFunctionType

    nc = bacc.Bacc()
    for v in (1e-30, -3.0, -2.0, -4.0):
        t_ = nc.alloc_sbuf_tensor(f"constx-{v}", [128, 1], f32)
        nc.gpsimd.memset(t_.ap(), v)
        nc.const_aps.aps[(f32, v)] = t_.ap()
    nc.all_engine_barrier()
    imgs_d = nc.declare_dram_parameter("imgs", [IPC * 3, P, FDTOT], f32, isOutput=False)
    scal_d = nc.declare_dram_parameter("scal", [P, 4 * IPC], f32, isOutput=False)
    out_d = nc.declare_dram_parameter("out", [IPC * 3, P, FDTOT], f32, isOutput=True)

    with TileContext(nc) as tc:
        with tc.tile_pool(name="const", bufs=1) as cpool, \
             tc.tile_pool(name="work", bufs=2) as pool:
            scal_ld = cpool.tile([P, 4 * IPC], f32, name="scal_ld")
            scal_sb = cpool.tile([P, 4 * IPC], f32, name="scal_sb")
            nc.sync.dma_start(out=scal_ld[:, :], in_=scal_d[:, :])
            nc.vector.tensor_copy(scal_sb[:, :], scal_ld[:, :])

            for img in range(IPC):
                ds_ap = scal_sb[:, 4 * img + 0:4 * img + 1]
                h6_ap = scal_sb[:, 4 * img + 1:4 * img + 2]
                h8_ap = scal_sb[:, 4 * img + 2:4 * img + 3]
                h10_ap = scal_sb[:, 4 * img + 3:4 * img + 4]
                for chk in range(NCH):
                    lo = chk * FD
                    in3 = pool.tile([P, 3, FD], f32, tag="in3", name="in3")
                    nc.sync.dma_start(
                        out=in3[:, :, :],
                        in_=imgs_d[3 * img:3 * img + 3, :, lo:lo + FD].rearrange("c p f -> p c f"))
                    r, g, b = in3[:, 0, :], in3[:, 1, :], in3[:, 2, :]
                    out3 = pool.tile([P, 3, FD], f32, tag="out3", name="out3")

                    t = lambda tag: pool.tile([P, FD], f32, tag=tag, name=tag)
                    maxc = t("maxc"); minc = t("minc"); cr = t("cr")
                    lncr = t("lncr"); icr = t("icr")
                    c = t("c"); d1 = t("d1"); num = t("num"); d2 = t("d2"); d3 = t("d3")
                    isg = pool.tile([P, FD], mybir.dt.uint32, tag="isg", name="isg")
                    isb = pool.tile([P, FD], mybir.dt.uint32, tag="isb", name="isb")
                    q2 = t("q2"); q3 = t("q3")
                    u = t("u"); m = t("m"); p = t("p")
                    mxt = t("mxt"); mnt = t("mnt")

                    # chroma
                    nc.vector.tensor_tensor(mxt[:, :], r, g, Alu.max)
                    nc.vector.tensor_tensor(maxc[:, :], mxt[:, :], b, Alu.max)
                    nc.vector.tensor_tensor(mnt[:, :], r, g, Alu.min)
                    nc.vector.tensor_tensor(minc[:, :], mnt[:, :], b, Alu.min)
                    nc.vector.tensor_tensor(cr[:, :], maxc[:, :], minc[:, :], Alu.subtract)

                    # 1/cr via ln/exp on ScalarE
                    nc.scalar.activation(lncr[:, :], cr[:, :], Act.Ln, bias=1e-30)
                    nc.scalar.activation(icr[:, :], lncr[:, :], Act.Exp, scale=-1.0)

                    # c = min(cr*ds, maxc) ; p = v - c
                    nc.vector.scalar_tensor_tensor(c[:, :], cr[:, :], ds_ap, maxc[:, :], Alu.mult, Alu.min)
                    nc.vector.tensor_tensor(p[:, :], maxc[:, :], c[:, :], Alu.subtract)

                    # hue numerator with branch priority r > g > b
                    nc.vector.tensor_tensor(d1[:, :], g, b, Alu.subtract)
                    nc.gpsimd.tensor_tensor(d2[:, :], b, r, Alu.subtract)
                    nc.gpsimd.tensor_tensor(d3[:, :], r, g, Alu.subtract)
                    nc.vector.scalar_tensor_tensor(num[:, :], cr[:, :], h6_ap, d1[:, :], Alu.mult, Alu.add)
                    nc.vector.scalar_tensor_tensor(q2[:, :], cr[:, :], h8_ap, d2[:, :], Alu.mult, Alu.add)
                    nc.vector.scalar_tensor_tensor(q3[:, :], cr[:, :], h10_ap, d3[:, :], Alu.mult, Alu.add)
                    nc.vector.tensor_tensor(isg[:, :], g, maxc[:, :], Alu.is_equal)
                    nc.vector.tensor_tensor(isb[:, :], b, maxc[:, :], Alu.is_equal)
                    nc.vector.copy_predicated(num[:, :], isg[:, :], q2[:, :])
                    nc.vector.copy_predicated(num[:, :], isb[:, :], q3[:, :])

                    # u = num/cr in [2,14]; m = mod(u, 6) in [0,6)
                    nc.vector.tensor_tensor(u[:, :], num[:, :], icr[:, :], Alu.mult)
                    if POOL_MOD:
                        nc.gpsimd.tensor_scalar(m[:, :], u[:, :], 6.0, None, Alu.mod)
                    else:
                        f1 = t("f1")
                        nc.vector.tensor_scalar(f1[:, :], u[:, :], 6.0, -6.0, Alu.is_ge, Alu.mult)
                        nc.vector.tensor_tensor(m[:, :], u[:, :], f1[:, :], Alu.add)
                        f2 = t("f2")
                        nc.vector.tensor_scalar(f2[:, :], m[:, :], 6.0, -6.0, Alu.is_ge, Alu.mult)
                        nc.vector.tensor_tensor(m[:, :], m[:, :], f2[:, :], Alu.add)

                    # per-channel triangle: out = v -/+ clamp((|m-k|-1)*c, 0, c)
                    for ch, (k, addsub) in enumerate(((3.0, True), (2.0, False), (4.0, False))):
                        a = t(f"a{ch}")
                        w = t(f"w{ch}"); x = t(f"x{ch}")
                        o = out3[:, ch, :]
                        nc.scalar.activation(a[:, :], m[:, :], Act.Abs, bias=-k)
                        nc.vector.scalar_tensor_tensor(w[:, :], a[:, :], 1.0, c[:, :], Alu.subtract, Alu.mult)
                        nc.vector.scalar_tensor_tensor(x[:, :], w[:, :], 0.0, c[:, :], Alu.max, Alu.min)
                        if addsub:
                            nc.vector.tensor_tensor(o, p[:, :], x[:, :], Alu.add)
                        else:
                            nc.vector.tensor_tensor(o, maxc[:, :], x[:, :], Alu.subtract)
                    nc.sync.dma_start(
                        out=out_d[3 * img:3 * img + 3, :, lo:lo + FD].rearrange("c p f -> p c f"),
                        in_=out3[:, :, :])
    nc.finalize()
    return nc


def _make_in_maps(imgs: np.ndarray, xf: np.ndarray):
    in_maps = []
    for core in range(N_CORES):
        sl = slice(core * IPC, (core + 1) * IPC)
        shard = imgs[sl].reshape(IPC * 3, P, FDTOT)
        hue = xf[sl, 0]
        sat = xf[sl, 1]
        scal = np.empty((P, 4 * IPC), dtype=np.float32)
        scal[:, 0::4] = sat[None, :]
        scal[:, 1::4] = (6.0 * hue + 6.0)[None, :]
        scal[:, 2::4] = (6.0 * hue + 8.0)[None, :]
        scal[:, 3::4] = (6.0 * hue + 10.0)[None, :]
        in_maps.append({"imgs": shard, "scal": scal})
    return in_maps


def kernel(imgs: np.ndarray, xform_params: np.ndarray) -> np.ndarray:
    from concourse.bass_utils import run_bass_kernel_spmd

    if "nc" not in _nc_cache:
        _nc_cache["nc"] = _build_nc()
    nc = _nc_cache["nc"]

    imgs = np.ascontiguousarray(imgs, dtype=np.float32)
    xf = np.asarray(xform_params, dtype=np.float32)

    in_maps = _make_in_maps(imgs, xf)
    res = run_bass_kernel_spmd(nc, in_maps, core_ids=list(range(N_CORES)))
    out = np.empty((B, C, H, W), dtype=np.float32)
    for core in range(N_CORES):
        out[core * IPC:(core + 1) * IPC] = res.results[core]["out"].reshape(IPC, C, H, W)
    return out



# revision 5
# speedup vs baseline: 1.8126x; 1.8126x over previous
"""AdjustHueSaturation Trainium2 kernel (fp16 pipeline).

Full inputs: imgs (64,3,512,512) f32 in [0,1], xform_params (64,2) f32
(hue delta in [-0.5,0.5], sat scale in [0.2,2]).  Output: (64,3,512,512) f32.

Sharding: pure batch data-parallel across 8 NeuronCores (8 images/core).
The host converts images to fp16 before upload and back to f32 after
download (halves HBM traffic; fp16 keeps worst-case error ~5e-3, well
under the 2e-2 gate — validated in proto.py against the jax reference).

Per-pixel math (per image, host precomputes ds and A'_j):
    A'_j = mod(6*dh + 2j + 3, 6) - 3   in [-3,3),  j=0,1,2 (r,g,b branch)
    maxc, minc, cr = max, min, chroma
    icr  = exp(-ln(cr + 3e-5))          # ScalarE Ln/Exp (one table set)
    c    = min(cr*ds, maxc); p = maxc - c
    d1=g-b, d2=b-r, d3n=g-r
    n    = cr*A'_0 + d1, overwritten by cr*A'_1 + d2 where g>r,
           then by cr*A'_2 - d3n where b>max(r,g)      # hue branch select
    w    = n*icr in [-4,4];  m6 = w + 3 - 6*[w>=0] in [-3,3)
    x_ch = clamp((|m6 + k_ch| - 1)*c, 0, c), k = 0/+1/-1   # custom DVE op
    out  = (p + x_r, maxc - x_g, maxc - x_b)
Matches the reference up to fp16 rounding (branch ties are continuous).
"""

import numpy as np

B, C, H, W = 64, 3, 512, 512
N_CORES = 8
IPC = B // N_CORES          # images per core
P = 128                     # SBUF partitions
FDTOT = (H * W) // P        # 2048 elems per partition per plane
FD = 1024                   # free-dim chunk per tile
NCH = FDTOT // FD

GPS_MIN = False              # min-tree on GpSimd
GPS_D2 = False              # d2 on GpSimd

_cache = {}


def _tri_op():
    """x = clamp((|in0 + s0| - 1) * in1, 0, in1) as one custom DVE op."""
    if "tri" in _cache:
        return _cache["tri"]
    import concourse.dve_ops as dvo
    from concourse.dve_spec import Spec, Src0, Src1, C0, Zero, One, maxx, minn, lower
    from concourse.dve_ops import DveOp, DveOpSpec

    t1 = Src0 + C0
    a = maxx(t1, Zero - t1)
    w = (a - One) * Src1
    body = minn(maxx(w, Zero), Src1)
    spec = Spec(
        body=body,
        reference=lambda in0, in1, s0: np.minimum(
            np.maximum((np.abs(in0 + s0) - 1.0) * in1, 0.0), in1
        ),
    )
    shas = {}
    for ver in ("v3", "v4"):
        shas[ver] = DveOpSpec(
            name="HSV_TRI", opcode=0, uops=lower(spec, ver=ver), rd1_en=True
        ).sha(ver)
    op = DveOp("HSV_TRI", spec, subdim=False, uops_sha=shas)
    dvo.OPS.append(op)
    dvo.CUSTOM_DVE_SPECS[op.name] = op.spec
    dvo._SUB_OPCODE_FOR_NAME[op.name] = dvo._CUSTOM_DVE_ROW_BASE + len(dvo.OPS) - 1
    assert dvo._SUB_OPCODE_FOR_NAME[op.name] < 0x20
    _cache["tri"] = op
    return op


def _build_nc():
    from concourse import bass, bacc, mybir
    from concourse.tile import TileContext

    tri = _tri_op()

    f16 = mybir.dt.float16
    f32 = mybir.dt.float32
    Alu = mybir.AluOpType
    Act = mybir.ActivationFunctionType

    nc = bacc.Bacc()
    # const AP for the Ln bias (activation converts float bias to a const AP)
    t_ = nc.alloc_sbuf_tensor("const-lnbias", [128, 1], f32)
    nc.gpsimd.memset(t_.ap(), 3e-5)
    nc.const_aps.aps[(f32, 3e-5)] = t_.ap()
    nc.all_engine_barrier()

    imgs_d = nc.declare_dram_parameter("imgs", [IPC * 3, P, FDTOT], f16, isOutput=False)
    scal_d = nc.declare_dram_parameter("scal", [P, 4 * IPC], f16, isOutput=False)
    out_d = nc.declare_dram_parameter("out", [IPC * 3, P, FDTOT], f16, isOutput=True)

    with TileContext(nc) as tc:
        with tc.tile_pool(name="const", bufs=1) as cpool, \
             tc.tile_pool(name="work", bufs=2) as pool:
            scal_ld = cpool.tile([P, 4 * IPC], f16, name="scal_ld")
            scal_sb = cpool.tile([P, 4 * IPC], f16, name="scal_sb")
            nc.sync.dma_start(out=scal_ld[:, :], in_=scal_d[:, :])
            nc.vector.tensor_copy(scal_sb[:, :], scal_ld[:, :])

            for img in range(IPC):
                ds_ap = scal_sb[:, 4 * img + 0:4 * img + 1]
                a0_ap = scal_sb[:, 4 * img + 1:4 * img + 2]
                a1_ap = scal_sb[:, 4 * img + 2:4 * img + 3]
                a2_ap = scal_sb[:, 4 * img + 3:4 * img + 4]
                for chk in range(NCH):
                    lo = chk * FD
                    in3 = pool.tile([P, 3, FD], f16, tag="in3", name="in3")
                    nc.sync.dma_start(
                        out=in3[:, :, :],
                        in_=imgs_d[3 * img:3 * img + 3, :, lo:lo + FD].rearrange("c p f -> p c f"))
                    r, g, b = in3[:, 0, :], in3[:, 1, :], in3[:, 2, :]
                    out3 = pool.tile([P, 3, FD], f16, tag="out3", name="out3")

                    def t(tag, dt=f16):
                        return pool.tile([P, FD], dt, tag=tag, name=tag)

                    mxt = t("mxt"); maxc = t("maxc"); mnt = t("mnt"); minc = t("minc")
                    cr = t("cr"); lncr = t("lncr", f32); icr = t("icr")
                    c = t("c"); p = t("p")
                    d1 = t("d1"); d2 = t("d2"); d3n = t("d3n")
                    m1k = t("m1k", mybir.dt.uint16); m2k = t("m2k", mybir.dt.uint16)
                    n1 = t("n1"); n2 = t("n2"); n3 = t("n3")
                    w = t("w"); f2 = t("f2"); m6 = t("m6")
                    x_r = t("x_r"); x_g = t("x_g"); x_b = t("x_b")

                    # chroma
                    nc.vector.tensor_tensor(mxt[:, :], r, g, Alu.max)
                    nc.vector.tensor_tensor(maxc[:, :], mxt[:, :], b, Alu.max)
                    eng_min = nc.gpsimd if GPS_MIN else nc.vector
                    eng_min.tensor_tensor(mnt[:, :], r, g, Alu.min)
                    eng_min.tensor_tensor(minc[:, :], mnt[:, :], b, Alu.min)
                    nc.vector.tensor_tensor(cr[:, :], maxc[:, :], minc[:, :], Alu.subtract)

                    # 1/cr on ScalarE (Ln out fp32 to keep icr accurate)
                    nc.scalar.activation(lncr[:, :], cr[:, :], Act.Ln, bias=3e-5)
                    nc.scalar.activation(icr[:, :], lncr[:, :], Act.Exp, scale=-1.0)

                    # c = min(cr*ds, maxc); p = maxc - c
                    nc.vector.scalar_tensor_tensor(c[:, :], cr[:, :], ds_ap, maxc[:, :], Alu.mult, Alu.min)
                    nc.vector.tensor_tensor(p[:, :], maxc[:, :], c[:, :], Alu.subtract)

                    # hue numerator with branch priority r < g < b overwrite
                    nc.vector.tensor_tensor(d1[:, :], g, b, Alu.subtract)
                    eng_d2 = nc.gpsimd if GPS_D2 else nc.vector
                    eng_d2.tensor_tensor(d2[:, :], b, r, Alu.subtract)
                    nc.vector.tensor_tensor(d3n[:, :], g, r, Alu.subtract)
                    nc.vector.tensor_scalar(m1k[:, :], d3n[:, :], 0.0, None, Alu.is_gt)
                    nc.vector.tensor_tensor(m2k[:, :], b, mxt[:, :], Alu.is_gt)
                    nc.vector.scalar_tensor_tensor(n1[:, :], cr[:, :], a0_ap, d1[:, :], Alu.mult, Alu.add)
                    nc.vector.scalar_tensor_tensor(n2[:, :], cr[:, :], a1_ap, d2[:, :], Alu.mult, Alu.add)
                    nc.vector.scalar_tensor_tensor(n3[:, :], cr[:, :], a2_ap, d3n[:, :], Alu.mult, Alu.subtract)
                    nc.vector.copy_predicated(n1[:, :], m1k[:, :], n2[:, :])
                    nc.vector.copy_predicated(n1[:, :], m2k[:, :], n3[:, :])

                    # w = n/cr in [-4,4]; m6 = w + 3 - 6*[w>=0] in [-3,3)
                    nc.vector.tensor_tensor(w[:, :], n1[:, :], icr[:, :], Alu.mult)
                    nc.vector.tensor_scalar(f2[:, :], w[:, :], 0.0, -6.0, Alu.is_ge, Alu.mult)
                    nc.vector.scalar_tensor_tensor(m6[:, :], w[:, :], 3.0, f2[:, :], Alu.add, Alu.add)

                    # x = clamp((|m6+k|-1)*c, 0, c), fused custom DVE op
                    nc.vector._custom_dve(tri, out=x_r[:, :], in0=m6[:, :], in1=c[:, :], s0=0.0)
                    nc.vector._custom_dve(tri, out=x_g[:, :], in0=m6[:, :], in1=c[:, :], s0=1.0)
                    nc.vector._custom_dve(tri, out=x_b[:, :], in0=m6[:, :], in1=c[:, :], s0=-1.0)

                    nc.vector.tensor_tensor(out3[:, 0, :], p[:, :], x_r[:, :], Alu.add)
                    nc.vector.tensor_tensor(out3[:, 1, :], maxc[:, :], x_g[:, :], Alu.subtract)
                    nc.vector.tensor_tensor(out3[:, 2, :], maxc[:, :], x_b[:, :], Alu.subtract)

                    nc.sync.dma_start(
                        out=out_d[3 * img:3 * img + 3, :, lo:lo + FD].rearrange("c p f -> p c f"),
                        in_=out3[:, :, :])
    nc.finalize()
    return nc


def _make_in_maps(imgs: np.ndarray, xf: np.ndarray):
    imgs16 = imgs.astype(np.float16)
    dh = xf[:, 0].astype(np.float64)
    sat = xf[:, 1].astype(np.float16)
    A = [(np.mod(6.0 * dh + 2 * j + 3, 6.0) - 3.0).astype(np.float16) for j in range(3)]
    in_maps = []
    for core in range(N_CORES):
        sl = slice(core * IPC, (core + 1) * IPC)
        shard = imgs16[sl].reshape(IPC * 3, P, FDTOT)
        scal = np.empty((P, 4 * IPC), dtype=np.float16)
        scal[:, 0::4] = sat[sl][None, :]
        scal[:, 1::4] = A[0][sl][None, :]
        scal[:, 2::4] = A[1][sl][None, :]
        scal[:, 3::4] = A[2][sl][None, :]
        in_maps.append({"imgs": shard, "scal": scal})
    return in_maps


def kernel(imgs: np.ndarray, xform_params: np.ndarray) -> np.ndarray:
    from concourse.bass_utils import run_bass_kernel_spmd

    if "nc" not in _cache:
        _cache["nc"] = _build_nc()
    nc = _cache["nc"]

    imgs = np.ascontiguousarray(imgs, dtype=np.float32)
    xf = np.asarray(xform_params, dtype=np.float32)

    in_maps = _make_in_maps(imgs, xf)
    res = run_bass_kernel_spmd(nc, in_maps, core_ids=list(range(N_CORES)))
    out = np.empty((B, C, H, W), dtype=np.float32)
    for core in range(N_CORES):
        out[core * IPC:(core + 1) * IPC] = (
            res.results[core]["out"].astype(np.float32).reshape(IPC, C, H, W))
    return out


# revision 12
# speedup vs baseline: 2.2086x; 1.2184x over previous
"""AdjustHueSaturation Trainium2 kernel (fp16 pipeline).

Full inputs: imgs (64,3,512,512) f32 in [0,1], xform_params (64,2) f32
(hue delta in [-0.5,0.5], sat scale in [0.2,2]).  Output: (64,3,512,512) f32.

Sharding: pure batch data-parallel across 8 NeuronCores (8 images/core).
The host converts images to fp16 before upload and back to f32 after
download (halves HBM traffic; fp16 keeps worst-case error ~5e-3, well
under the 2e-2 gate — validated in proto.py against the jax reference).

Per-pixel math (per image, host precomputes ds and A'_j):
    A'_j = mod(6*dh + 2j + 3, 6) - 3   in [-3,3),  j=0,1,2 (r,g,b branch)
    maxc, minc, cr = max, min, chroma
    icr  = exp(-ln(cr + 3e-5))          # ScalarE Ln/Exp (one table set)
    c    = min(cr*ds, maxc); p = maxc - c
    d1=g-b, d2=b-r, d3n=g-r
    n    = cr*A'_0 + d1, overwritten by cr*A'_1 + d2 where g>r,
           then by cr*A'_2 - d3n where b>max(r,g)      # hue branch select
    w    = n*icr in [-4,4];  m6 = w + 3 - 6*[w>=0] in [-3,3)
    x_ch = clamp((|m6 + k_ch| - 1)*c, 0, c), k = 0/+1/-1   # custom DVE op
    out  = (p + x_r, maxc - x_g, maxc - x_b)
Matches the reference up to fp16 rounding (branch ties are continuous).
"""

import numpy as np

B, C, H, W = 64, 3, 512, 512
N_CORES = 8
IPC = B // N_CORES          # images per core
P = 128                     # SBUF partitions
FDTOT = (H * W) // P        # 2048 elems per partition per plane
FD = 1024                   # free-dim chunk per tile
NCH = FDTOT // FD

GPS_MIN = False              # min-tree on GpSimd
GPS_D2 = False              # d2 on GpSimd

_cache = {}


def _register_op(name, spec):
    import concourse.dve_ops as dvo
    from concourse.dve_spec import lower, spec_leaves, Src1
    from concourse.dve_ops import DveOp, DveOpSpec, has_src1

    rd1 = has_src1(spec)
    shas = {}
    for ver in ("v3", "v4"):
        shas[ver] = DveOpSpec(
            name=name, opcode=0, uops=lower(spec, ver=ver), rd1_en=rd1
        ).sha(ver)
    op = DveOp(name, spec, subdim=False, uops_sha=shas)
    dvo.OPS.append(op)
    dvo.CUSTOM_DVE_SPECS[op.name] = op.spec
    dvo._SUB_OPCODE_FOR_NAME[op.name] = dvo._CUSTOM_DVE_ROW_BASE + len(dvo.OPS) - 1
    assert dvo._SUB_OPCODE_FOR_NAME[op.name] < 0x20
    return op


def _custom_ops():
    """Register the fused DVE ops (once per process)."""
    if "ops" in _cache:
        return _cache["ops"]
    from concourse.dve_spec import Spec, Src0, Src1, C0, C1, Zero, One, maxx, minn, select

    def _tri(x, k):
        tk = x + k
        a = maxx(tk, Zero - tk)
        return minn(maxx((a - One) * Src1, Zero), Src1)

    # x = clamp((|m6 + s0| - 1) * c, 0, c)
    tri = _register_op("HSV_TRI", Spec(
        body=_tri(Src0, C0),
        reference=lambda in0, in1, s0: np.minimum(
            np.maximum((np.abs(in0 + s0) - 1.0) * in1, 0.0), in1),
    ))
    # c - clamp(...) (red channel: o_r = maxc - (c - x_r))
    tri_r = _register_op("HSV_TRI_R", Spec(
        body=Src1 - _tri(Src0, C0),
        reference=lambda in0, in1, s0: in1 - np.minimum(
            np.maximum((np.abs(in0 + s0) - 1.0) * in1, 0.0), in1),
    ))
    # m6 = wrap(n*icr): w = in0*in1 in [-4,4]; m6 = w + (w>=0 ? s0 : s1)
    wrapmul = _register_op("HSV_WRAPMUL", Spec(
        body=(lambda w: w + select(w >= Zero, C0, C1))(Src0 * Src1),
        reference=lambda in0, in1, s0, s1: (lambda w: w + np.where(w >= 0, s0, s1))(in0 * in1),
    ))
    _cache["ops"] = (tri, tri_r, wrapmul)
    return _cache["ops"]


def _patch_act_tables():
    """Force Ln+Exp to resolve to the one table set containing both, so the
    table load hoists out of the loop instead of thrashing every iteration."""
    if _cache.get("act_patched"):
        return
    import concourse.bacc as bacc_mod
    orig = bacc_mod.get_activation_tables

    def patched(arch):
        tables = orig(arch)
        keep = "natural_log_exp_and_others"
        out = {}
        for name, fns in tables.items():
            if name != keep:
                fns = {f for f in fns if str(getattr(f, "name", f)).lower() not in ("ln", "exp")}
            out[name] = fns
        return out

    bacc_mod.get_activation_tables = patched
    _cache["act_patched"] = True


def _build_nc():
    from concourse import bass, bacc, mybir
    from concourse.tile import TileContext

    tri, tri_r, wrapmul = _custom_ops()
    _patch_act_tables()

    f16 = mybir.dt.float16
    f32 = mybir.dt.float32
    Alu = mybir.AluOpType
    Act = mybir.ActivationFunctionType

    nc = bacc.Bacc()
    # const AP for the Ln bias (activation converts float bias to a const AP)
    t_ = nc.alloc_sbuf_tensor("const-lnbias", [128, 1], f32)
    nc.gpsimd.memset(t_.ap(), 3e-5)
    nc.const_aps.aps[(f32, 3e-5)] = t_.ap()
    nc.all_engine_barrier()

    imgs_d = nc.declare_dram_parameter("imgs", [IPC * 3, P, FDTOT], f16, isOutput=False)
    scal_d = nc.declare_dram_parameter("scal", [P, 4 * IPC], f32, isOutput=False)
    out_d = nc.declare_dram_parameter("out", [IPC * 3, P, FDTOT], f16, isOutput=True)

    with TileContext(nc) as tc:
        with tc.tile_pool(name="const", bufs=1) as cpool, \
             tc.tile_pool(name="work", bufs=2) as pool:
            scal_ld = cpool.tile([P, 4 * IPC], f32, name="scal_ld")
            scal_sb = cpool.tile([P, 4 * IPC], f32, name="scal_sb")
            nc.sync.dma_start(out=scal_ld[:, :], in_=scal_d[:, :])
            nc.vector.tensor_copy(scal_sb[:, :], scal_ld[:, :])

            for img in range(IPC):
                ds_ap = scal_sb[:, 4 * img + 0:4 * img + 1]
                a0_ap = scal_sb[:, 4 * img + 1:4 * img + 2]
                a1_ap = scal_sb[:, 4 * img + 2:4 * img + 3]
                a2_ap = scal_sb[:, 4 * img + 3:4 * img + 4]
                for chk in range(NCH):
                    lo = chk * FD
                    in3 = pool.tile([P, 3, FD], f16, tag="in3", name="in3")
                    nc.sync.dma_start(
                        out=in3[:, :, :],
                        in_=imgs_d[3 * img:3 * img + 3, :, lo:lo + FD].rearrange("c p f -> p c f"))
                    r, g, b = in3[:, 0, :], in3[:, 1, :], in3[:, 2, :]
                    out3 = pool.tile([P, 3, FD], f16, tag="out3", name="out3")

                    def t(tag, dt=f16):
                        return pool.tile([P, FD], dt, tag=tag, name=tag)

                    mxt = t("mxt"); maxc = t("maxc"); mnt = t("mnt"); minc = t("minc")
                    cr = t("cr"); lncr = t("lncr", f32); icr = t("icr")
                    crds = t("crds"); c = t("c")
                    crA0 = t("crA0"); crA1 = t("crA1"); crA2 = t("crA2")
                    d1 = t("d1"); d2 = t("d2"); d3n = t("d3n")
                    m1k = t("m1k", mybir.dt.uint16); m2k = t("m2k", mybir.dt.uint16)
                    n1 = t("n1"); n2 = t("n2"); n3 = t("n3")
                    m6 = t("m6")
                    x_r = t("x_r"); x_g = t("x_g"); x_b = t("x_b")

                    # chroma (DVE)
                    nc.vector.tensor_tensor(mxt[:, :], r, g, Alu.max)
                    nc.vector.tensor_tensor(maxc[:, :], mxt[:, :], b, Alu.max)
                    nc.vector.tensor_tensor(mnt[:, :], r, g, Alu.min)
                    nc.vector.tensor_tensor(minc[:, :], mnt[:, :], b, Alu.min)
                    nc.vector.tensor_tensor(cr[:, :], maxc[:, :], minc[:, :], Alu.subtract)

                    # ScalarE: 1/cr (Ln out fp32) + the per-image scalar multiplies
                    nc.scalar.activation(lncr[:, :], cr[:, :], Act.Ln, bias=3e-5)
                    nc.scalar.activation(icr[:, :], lncr[:, :], Act.Exp, scale=-1.0)
                    nc.scalar.activation(crds[:, :], cr[:, :], Act.Copy, scale=ds_ap)
                    nc.scalar.activation(crA0[:, :], cr[:, :], Act.Copy, scale=a0_ap)
                    nc.scalar.activation(crA1[:, :], cr[:, :], Act.Copy, scale=a1_ap)
                    nc.scalar.activation(crA2[:, :], cr[:, :], Act.Copy, scale=a2_ap)

                    nc.vector.tensor_tensor(c[:, :], crds[:, :], maxc[:, :], Alu.min)

                    # hue numerator, branch priority r < g < b overwrite
                    nc.vector.tensor_tensor(d1[:, :], g, b, Alu.subtract)
                    nc.vector.tensor_tensor(d2[:, :], b, r, Alu.subtract)
                    nc.vector.tensor_tensor(d3n[:, :], g, r, Alu.subtract)
                    nc.vector.tensor_scalar(m1k[:, :], d3n[:, :], 0.0, None, Alu.is_gt)
                    nc.vector.tensor_tensor(m2k[:, :], b, mxt[:, :], Alu.is_gt)
                    nc.vector.tensor_tensor(n1[:, :], crA0[:, :], d1[:, :], Alu.add)
                    nc.vector.tensor_tensor(n2[:, :], crA1[:, :], d2[:, :], Alu.add)
                    nc.vector.tensor_tensor(n3[:, :], crA2[:, :], d3n[:, :], Alu.subtract)
                    nc.vector.copy_predicated(n1[:, :], m1k[:, :], n2[:, :])
                    nc.vector.copy_predicated(n1[:, :], m2k[:, :], n3[:, :])

                    # m6 = wrap(n/cr) in [-3,3): fused mult + fold
                    nc.vector._custom_dve(wrapmul, out=m6[:, :], in0=n1[:, :], in1=icr[:, :], s0=-3.0, s1=3.0)

                    # x = clamp((|m6+k|-1)*c, 0, c), fused; red emits c-x
                    nc.vector._custom_dve(tri_r, out=x_r[:, :], in0=m6[:, :], in1=c[:, :], s0=0.0)
                    nc.vector._custom_dve(tri, out=x_g[:, :], in0=m6[:, :], in1=c[:, :], s0=1.0)
                    nc.vector._custom_dve(tri, out=x_b[:, :], in0=m6[:, :], in1=c[:, :], s0=-1.0)

                    nc.vector.tensor_tensor(out3[:, 0, :], maxc[:, :], x_r[:, :], Alu.subtract)
                    nc.vector.tensor_tensor(out3[:, 1, :], maxc[:, :], x_g[:, :], Alu.subtract)
                    nc.vector.tensor_tensor(out3[:, 2, :], maxc[:, :], x_b[:, :], Alu.subtract)

                    nc.sync.dma_start(
                        out=out_d[3 * img:3 * img + 3, :, lo:lo + FD].rearrange("c p f -> p c f"),
                        in_=out3[:, :, :])
    nc.finalize()
    return nc


def _make_in_maps(imgs: np.ndarray, xf: np.ndarray):
    imgs16 = imgs.astype(np.float16)
    dh = xf[:, 0].astype(np.float64)
    sat = xf[:, 1].astype(np.float32)
    A = [(np.mod(6.0 * dh + 2 * j + 3, 6.0) - 3.0).astype(np.float32) for j in range(3)]
    in_maps = []
    for core in range(N_CORES):
        sl = slice(core * IPC, (core + 1) * IPC)
        shard = imgs16[sl].reshape(IPC * 3, P, FDTOT)
        scal = np.empty((P, 4 * IPC), dtype=np.float32)
        scal[:, 0::4] = sat[sl][None, :]
        scal[:, 1::4] = A[0][sl][None, :]
        scal[:, 2::4] = A[1][sl][None, :]
        scal[:, 3::4] = A[2][sl][None, :]
        in_maps.append({"imgs": shard, "scal": scal})
    return in_maps


def kernel(imgs: np.ndarray, xform_params: np.ndarray) -> np.ndarray:
    from concourse.bass_utils import run_bass_kernel_spmd

    if "nc" not in _cache:
        _cache["nc"] = _build_nc()
    nc = _cache["nc"]

    imgs = np.ascontiguousarray(imgs, dtype=np.float32)
    xf = np.asarray(xform_params, dtype=np.float32)

    in_maps = _make_in_maps(imgs, xf)
    res = run_bass_kernel_spmd(nc, in_maps, core_ids=list(range(N_CORES)))
    out = np.empty((B, C, H, W), dtype=np.float32)
    for core in range(N_CORES):
        out[core * IPC:(core + 1) * IPC] = (
            res.results[core]["out"].astype(np.float32).reshape(IPC, C, H, W))
    return out
